# revision 1
# baseline (speedup 1.0000x reference)
"""Trainium2 Bass kernel for nn_Conv2DLinalgRMSNorm.

Math: out = RMSNormEps(x @ (sum_l conv_w[l])^T / 20) * norm_w
  where RMSNormEps(v) = v / sqrt(sum_h v^2 + eps*H) * sqrt(H)

Strategy (8 NeuronCores, no cross-core collectives):
  Launch 1 (weight prep, sharded over output-channel rows):
    core i reads conv_w[:, i*128:(i+1)*128, :] (10 MiB), sums the 20
    layers on DVE, PE-transposes the [128, 1024] row-block and writes
    its [1024, 128] column slice of W_sum^T, rounded to float32r.
  Host: concatenates the 8 slices into W^T [1024, 1024].
  Launch 2 (token-parallel GEMM + norm):
    core i takes 1024 tokens, PE-transposes x tiles on chip, GEMM in
    float32r (e8m11, 4x faster than fp32), RMSNorm fused on ACT/DVE.
    The 1/20 scaling folds into the rsqrt bias:
      out = y * 32 * rsqrt(sum y^2 + 400*eps*H) * norm_w,  y = x @ Wsum^T.
"""
import numpy as np

import concourse.bass as bass
import concourse.mybir as mybir
from concourse.tile import TileContext
from concourse import bass_utils

dt = mybir.dt
P = 128
H = 1024
NL = 20
B, S = 2, 4096
TOK = B * S            # 8192
NCORES = 8
TPC = TOK // NCORES    # 1024 tokens per core
EPS = 1e-6
SSQ_BIAS = float(NL * NL * EPS * H)   # 0.4096

_ctr = [0]


def _legalize_waits(nc):
    """This walrus build accepts 1 sync wait per instruction (2 on
    EventSemaphore); split excess waits into standalone waits."""
    def fix_block(blk):
        insts = list(blk.instructions)
        out = []
        changed = False
        for inst in insts:
            si = inst.sync_info
            waits = list(si.on_wait) if si and si.on_wait else []
            cap = 2 if isinstance(inst, mybir.InstEventSemaphore) else 1
            if len(waits) > cap:
                changed = True
                keep = waits[:cap]
                extra = waits[cap:]
                for i in range(0, len(extra), 2):
                    chunk = extra[i:i + 2]
                    _ctr[0] += 1
                    ev = mybir.InstEventSemaphore(
                        name=f"I-waitfix-{_ctr[0]}",
                        engine=inst.engine,
                        ins=[],
                        outs=[],
                        sync_info=mybir.SyncInfo(on_wait=chunk, on_update=[]),
                    )
                    out.append(ev)
                si.on_wait = keep
            out.append(inst)
        if changed:
            blk.instructions = out
        for sub in getattr(blk, "blocks", None) or []:
            fix_block(sub)

    for fn in nc.m.functions:
        for blk in fn.blocks:
            fix_block(blk)


def _make_identity(nc, identity):
    nc.gpsimd.memset(identity, 0.0)
    nc.gpsimd.affine_select(
        out=identity,
        in_=identity,
        compare_op=mybir.AluOpType.not_equal,
        fill=1.0,
        base=0,
        pattern=[[-1, identity.shape[0]]],
        channel_multiplier=1,
    )


def build_l1():
    """Weight prep: conv slice [20, 128, 1024] -> wt piece [1024, 128] f32r.

    Per-layer contiguous 512 KiB loads; the 20-layer sum is split between
    DVE (strided reduce over low h-half) and GpSimd (add chain over high
    h-half); the summed [128, 1024] block is PE-transposed in 8 blocks and
    written out rounded to float32r.
    """
    nc = bass.Bass('TRN2', target_bir_lowering=False, debug=False)
    cw = nc.dram_tensor("cw", [NL, P, H], dt.float32, kind="ExternalInput")
    wtp = nc.dram_tensor("wtp", [H, P], dt.float32r, kind="ExternalOutput")
    NCH = 8          # h blocks of 128
    NQ = 4           # pipeline chunks of 256 over h
    QW = H // NQ     # 256
    NDVE = 12        # layers summed on DVE; the rest accumulate on PE
    with TileContext(nc) as tc:
        with (
            tc.tile_pool(name="load", bufs=3) as load,
            tc.tile_pool(name="ws", bufs=1) as wsp,
            tc.tile_pool(name="psum", bufs=4, space="PSUM") as psum,
        ):
            ident = wsp.tile([P, P], dt.float32, tag="ident")
            _make_identity(nc, ident[:])
            # PE warm-up: dense dummy transposes release the HAM clock gate
            # (cold PE runs at 1.2 GHz) while the first DMA chunk streams in
            wu = psum.tile([P, P], dt.float32, tag="wu")
            for _ in range(14):
                nc.tensor.matmul(wu[:], ident[:], ident[:], is_transpose=True,
                                 start=True, stop=True)
            wt = wsp.tile([P, NCH, P], dt.float32r, tag="wt")
            dsum = wsp.tile([P, H], dt.float32, tag="dsum")
            for q in range(NQ):
                hsl = bass.ds(q * QW, QW)
                t = load.tile([P, NL, QW], dt.float32, tag="t")
                nc.sync.dma_start(t[:], cw.rearrange("l p h -> p l h")[:, :, hsl])
                # DVE sums the first NDVE layers via a contiguous add tree
                # (strided tensor_reduce measured ~2.7 cyc/elem; adds ~1)
                sc = load.tile([P, 6, QW], dt.float32, tag="sc")
                for i in range(6):
                    nc.vector.tensor_add(sc[:, i], t[:, 2 * i, :], t[:, 2 * i + 1, :])
                for i in range(3):
                    nc.vector.tensor_add(sc[:, i], sc[:, 2 * i], sc[:, 2 * i + 1])
                nc.vector.tensor_add(sc[:, 3], sc[:, 0], sc[:, 1])
                nc.vector.tensor_add(dsum[:, hsl], sc[:, 3], sc[:, 2])
                # PE transpose-accumulates the remaining layers + the DVE sum
                for b in range(QW // P):
                    hb = q * (QW // P) + b
                    bsl = bass.ds(b * P, P)
                    pt = psum.tile([P, P], dt.float32, tag="pt")
                    for i, l in enumerate(range(NDVE, NL)):
                        nc.tensor.matmul(
                            pt[:], t[:, l, bsl], ident[:], is_transpose=True,
                            start=(i == 0), stop=False,
                        )
                    nc.tensor.matmul(
                        pt[:], dsum[:, bass.ds(hb * P, P)], ident[:],
                        is_transpose=True, start=False, stop=True,
                    )
                    nc.vector.tensor_copy(wt[:, hb, :], pt[:])  # rounds to f32r
                    nc.sync.dma_start(
                        wtp.rearrange("(c h) o -> h c o", c=NCH)[:, hb, :],
                        wt[:, hb, :],
                    )
    _legalize_waits(nc)
    return nc


def build_l2():
    """Token shard GEMM + LinalgRMSNorm: x [1024, 1024], wt [1024, 1024] f32r."""
    nc = bass.Bass('TRN2', target_bir_lowering=False, debug=False)
    x = nc.dram_tensor("x", [TPC, H], dt.float32, kind="ExternalInput")
    wt = nc.dram_tensor("wt", [H, H], dt.float32r, kind="ExternalInput")
    nw = nc.dram_tensor("nw", [H], dt.float32, kind="ExternalInput")
    y = nc.dram_tensor("y", [TPC, H], dt.float32, kind="ExternalOutput")
    NCH = 8
    NT = TPC // P     # 8 token tiles
    with TileContext(nc) as tc:
        with (
            tc.tile_pool(name="w", bufs=1) as wp,
            tc.tile_pool(name="xin", bufs=4) as xin,
            tc.tile_pool(name="xt", bufs=3) as xtp,
            tc.tile_pool(name="yout", bufs=3) as yp,
            tc.tile_pool(name="sq", bufs=2) as sqp,
            tc.tile_pool(name="stat", bufs=6) as stat,
            tc.tile_pool(name="psum", bufs=3, space="PSUM") as psum,
            tc.tile_pool(name="psumT", bufs=2, space="PSUM") as psumT,
        ):
            # weights on the Activation HWDGE queue so token loads on the
            # SP queue aren't stuck behind 4 MiB of wt
            wt_sb = wp.tile([P, NCH, H], dt.float32r, tag="wt_sb")
            wt_r = wt.rearrange("(c p) o -> p c o", p=P)
            for hc in range(NCH):
                nc.scalar.dma_start(wt_sb[:, hc, :], wt_r[:, hc, :])
            nwb = wp.tile([P, H], dt.float32, tag="nwb")
            nc.scalar.dma_start(nwb[:], nw[None, :].partition_broadcast(P))
            ident = wp.tile([P, P], dt.float32, tag="ident")
            _make_identity(nc, ident[:])
            # PE warm-up while the first x tile loads (see build_l1)
            wu = psumT.tile([P, P], dt.float32, tag="ptr")
            for _ in range(14):
                nc.tensor.matmul(wu[:], ident[:], ident[:], is_transpose=True,
                                 start=True, stop=True)

            for tt in range(NT):
                tsl = bass.ds(tt * P, P)
                xrow = xin.tile([P, H], dt.float32, tag="xrow")
                nc.sync.dma_start(xrow[:], x[tsl, :])
                xT = xtp.tile([P, NCH, P], dt.float32r, tag="xT")
                for hc in range(NCH):
                    ptr = psumT.tile([P, P], dt.float32, tag="ptr")
                    nc.tensor.transpose(ptr[:], xrow[:, bass.ds(hc * P, P)], ident[:])
                    nc.vector.tensor_copy(xT[:, hc, :], ptr[:])  # rounds to f32r

                # one PSUM tile [128, 1024] (2 banks), two 8-matmul groups
                pt = psum.tile([P, H], dt.float32, tag="pt")
                for oh in range(2):
                    osl = bass.ds(oh * 512, 512)
                    for hc in range(NCH):
                        nc.tensor.matmul(
                            pt[:, osl], xT[:, hc, :], wt_sb[:, hc, osl],
                            start=(hc == 0), stop=(hc == NCH - 1),
                        )

                # ssq over the whole row on ACT (square + free-dim accum)
                sq = sqp.tile([P, H], dt.float32, tag="sq")
                v = stat.tile([P, 1], dt.float32, tag="v")
                nc.scalar.activation(
                    sq[:], pt[:], mybir.ActivationFunctionType.Square,
                    accum_out=v[:],
                )
                vb = stat.tile([P, 1], dt.float32, tag="vb")
                nc.vector.tensor_scalar(
                    vb[:], v[:], SSQ_BIAS, None, mybir.AluOpType.add,
                )
                rv = stat.tile([P, 1], dt.float32, tag="rv")
                nc.vector.reciprocal(rv[:], vb[:])
                s = stat.tile([P, 1], dt.float32, tag="s")
                nc.scalar.activation(
                    s[:], rv[:], mybir.ActivationFunctionType.Sqrt,
                    scale=float(H),
                )

                ysb = yp.tile([P, H], dt.float32, tag="ysb")
                nc.vector.scalar_tensor_tensor(
                    ysb[:], pt[:], s[:], nwb[:],
                    op0=mybir.AluOpType.mult, op1=mybir.AluOpType.mult,
                )
                nc.sync.dma_start(y[tsl, :], ysb[:])
    _legalize_waits(nc)
    return nc


_CACHE = {}


def _get(name, builder):
    if name not in _CACHE:
        _CACHE[name] = builder()
    return _CACHE[name]


def kernel(hidden_states, conv_w, norm_w):
    in_dtype = hidden_states.dtype
    x_flat = np.ascontiguousarray(
        np.asarray(hidden_states, dtype=np.float32).reshape(TOK, H)
    )
    conv_w = np.asarray(conv_w, dtype=np.float32)
    norm_w = np.asarray(norm_w, dtype=np.float32)
    core_ids = list(range(NCORES))

    # Launch 1: weight prep
    nc1 = _get("l1", build_l1)
    in1 = [
        {"cw": np.ascontiguousarray(conv_w[:, i * P:(i + 1) * P, :])}
        for i in range(NCORES)
    ]
    res1 = bass_utils.run_bass_kernel_spmd(nc1, in1, core_ids)
    wt_full = np.concatenate([res1.results[i]["wtp"] for i in range(NCORES)], axis=1)

    # Launch 2: GEMM + norm over token shards
    nc2 = _get("l2", build_l2)
    in2 = [
        {
            "x": np.ascontiguousarray(x_flat[i * TPC:(i + 1) * TPC]),
            "wt": wt_full,
            "nw": norm_w,
        }
        for i in range(NCORES)
    ]
    res2 = bass_utils.run_bass_kernel_spmd(nc2, in2, core_ids)
    y = np.concatenate([res2.results[i]["y"] for i in range(NCORES)], axis=0)
    return y.reshape(B, S, H).astype(in_dtype, copy=False)

